# revision 35
# baseline (speedup 1.0000x reference)
"""Cepstrum -> impulse response (Oppenheim recursion) on 8 Trainium2 cores.

Math: the recursion h[0]=exp(c[0]); h[n]=(1/n)*sum_m m*c[m]*h[n-m] is the
power-series exponential h = exp-series(c), so H(z)=exp(C(z)) and h[n]
decays super-exponentially (|h[128]| ~ 5e-3 max, tail norm beyond n=128 is
1.8e-3 of ||h||).  We therefore evaluate a K=128 *shifted-frequency* DFT
(w_k = 2*pi*(k+1/2)/K, k=0..63): for real input the 64 complex bins carry
everything with NO DC/Nyquist special cases (H_{K-1-k} = conj(H_k)), and
the inverse aliases with alternating sign, h_alias[n] = sum_j (-1)^j
h[n+jK], which is as tiny as the tail.  Columns 128..511 are exactly zero
on the host side.  Total rel err ~2.5e-3 incl. fp16 stages (tol 2e-2).

Per panel of 1024 batch rows (lanes 0:64 = rows 0:512, 64:128 = 512:1024):
  Cre = F_re^T @ cT   (PE, fp16)      E   = exp(Cre)       (ACT)
  Cim = F_im^T @ cT   (PE)            sin = Sin(Cim), cos = Sin(Cim+pi/2)
  ReH = E*cos, ImH = E*sin (DVE, fp16)
  hT  = Gre^T @ ReH + Gim^T @ ImH     (PE; G as weights, output [n, batch])
Output is written transposed [128, 8192] fp16 per core; the host
untransposes, upcasts and zero-pads to [B, 512] fp32.

Scheduling: exp and Sin live in different ACT tables (1283ns per reload),
so work is split in two halves of 2 input pairs and the ACT stream is
pinned to [Exp x4][Sin x4][Exp x4][Sin x4] (4 loads) with sim-only
tile_wait_until phases.  Cre PSUM tiles borrow idle ops/sps-pool slots so
the four exps of a half never serialize behind the 2-buffer cps rotation
(each hop costs ~1.8us of semaphore latency), PSUM->SBUF output casts are
split DVE/ACT-Copy (Copy needs no table), and the 8 PSUM banks are kept
exactly full: 2 cps + 4 Cim + 2 inverse-output.

Sharding: pure data parallel, batch 65536 -> 8 x 8192 rows.
"""

import math

import numpy as np

import concourse.bass as bass
import concourse.mybir as mybir
import concourse.tile as tile
from concourse.bass_utils import run_bass_kernel_spmd

F32 = mybir.dt.float32
F16 = mybir.dt.float16
AF = mybir.ActivationFunctionType

B_TOTAL = 65536
M1 = 100            # cepstral coeffs (order 99 + c0)
N_OUT = 512         # impulse response length
NCORES = 8
ROWS = B_TOTAL // NCORES    # 8192 rows per core

K_DFT = 128         # shifted-frequency DFT size; h[:, K_DFT:] == 0
NB = K_DFT // 2     # 64 complex bins
PANEL = 1024        # batch rows per panel (2 lane-halves of 512)
NPANEL = ROWS // PANEL      # 8
PAIR = 2 * PANEL    # input DMA granularity


def _split_multi_waits(nc):
    """walrus in this container rejects >1 sync-wait on a single instruction
    (setupSyncWait: 'Too many sync wait commands').  Move all but the last
    wait of every instruction onto preceding same-engine NoOps."""
    ctr = 0
    for f in nc.m.functions:
        for bb in f.blocks:
            out = []
            for ins in bb.instructions:
                si = ins.sync_info
                if si is not None and si.on_wait and len(si.on_wait) > 1:
                    waits = list(si.on_wait)
                    for w in waits[:-1]:
                        nop = mybir.InstNoOp(name=f"wsplit-{ctr}", ins=[], outs=[])
                        ctr += 1
                        nop.engine = ins.engine
                        nop.sync_info = mybir.SyncInfo(on_wait=[w], on_update=[])
                        out.append(nop)
                    si.on_wait = [waits[-1]]
                out.append(ins)
            if len(out) != len(bb.instructions):
                bb.instructions[:] = out
    return ctr


def _build_nc():
    nc = bass.Bass()
    c_in = nc.dram_tensor("c", [M1, ROWS], F16, kind="ExternalInput")
    fmat = nc.dram_tensor("fmat", [M1, 2, NB], F16, kind="ExternalInput")
    gmat = nc.dram_tensor("gmat", [128, 2, K_DFT], F16, kind="ExternalInput")
    h_out = nc.dram_tensor("h", [K_DFT, ROWS], F16, kind="ExternalOutput")

    with tile.TileContext(nc) as tc:
        with (
            tc.tile_pool(name="const", bufs=1) as constp,
            tc.tile_pool(name="cin", bufs=4) as cinp,
            tc.tile_pool(name="e", bufs=6) as epool,
            tc.tile_pool(name="trig", bufs=6) as trigp,
            tc.tile_pool(name="spec", bufs=6) as specp,
            tc.tile_pool(name="osb", bufs=4) as osbp,
            tc.tile_pool(name="sim", bufs=5) as simp,
            tc.tile_pool(name="cps", bufs=2, space="PSUM") as cpsp,
            tc.tile_pool(name="sps", bufs=2, space="PSUM") as spsp,
            tc.tile_pool(name="ops", bufs=2, space="PSUM") as opsp,
        ):
            f_sb = constp.tile([M1, 2, NB], F16)
            nc.scalar.dma_start(out=f_sb, in_=fmat[:, :, :])
            g_sb = constp.tile([128, 2, K_DFT], F16)
            nc.scalar.dma_start(out=g_sb, in_=gmat[:, :, :])
            halfpi = constp.tile([128, 1], F32)
            nc.vector.memset(halfpi, math.pi / 2)

            cts = {}

            def load_pair(q):
                # SP queue for inputs, ACT queue for consts: the first
                # matmul waits only its own queue's first completion
                ct2 = cinp.tile([M1, PAIR], F16, tag="ct2")
                nc.sync.dma_start(
                    out=ct2, in_=c_in[:, q * PAIR : (q + 1) * PAIR]
                )
                cts[q] = ct2

            def fwd_c_panel(p, pool):
                """Cre matmuls + exp for panel p; Cre tiles alternate
                between the cps and ops rings (4 buffers in flight) so the
                eight exps never serialize behind a 2-buffer rotation"""
                q = p // 2
                j = p % 2
                ct2 = cts[q]
                cps = pool.tile([128, 512], F32, tag="ops" if pool is opsp else "cps")
                for hp in range(2):
                    rhs = ct2[:, j * PANEL + hp * 512 : j * PANEL + (hp + 1) * 512]
                    nc.tensor.matmul(
                        cps[hp * 64 : hp * 64 + 64, :],
                        lhsT=f_sb[:, 0, :],
                        rhs=rhs,
                        start=True,
                        stop=True,
                    )
                nc.scalar.activation(
                    out=es[q][:, j, :], in_=cps, func=AF.Exp
                )

            def fwd_s_pair(q):
                """Cim matmuls for pair q; the f32 PSUM spectrum is parked
                in SBUF as fp16 immediately (DVE) so all four pairs' Cim
                coexist and every Sin can run after every Exp: 2 table
                loads total instead of 4"""
                ct2 = cts[q]
                s2 = spsp.tile([128, 2, 512], F32, tag="s2")
                for j in range(2):
                    for hp in range(2):
                        rhs = ct2[:, j * PANEL + hp * 512 : j * PANEL + (hp + 1) * 512]
                        nc.tensor.matmul(
                            s2[hp * 64 : hp * 64 + 64, j, :],
                            lhsT=f_sb[:, 1, :],
                            rhs=rhs,
                            start=True,
                            stop=True,
                        )
                sim = simp.tile([128, 2, 512], F16, tag="sim")
                nc.vector.tensor_copy(sim, s2)
                return sim

            def inv_pair(q, e_pair, s2, act_cast=False, extra_psum=False):
                """trig (Sin table), spectrum, inverse DFT, store for pair q"""
                sin2 = trigp.tile([128, 2, 512], F16, tag="sin")
                cos2 = trigp.tile([128, 2, 512], F16, tag="cos")
                nc.scalar.activation(out=sin2, in_=s2, func=AF.Sin)
                # cos(x) = sin(x + pi/2); |x| <= 1.62 so args stay in ACT
                # Sin's accurate range (-pi, pi)
                nc.scalar.activation(out=cos2, in_=s2, func=AF.Sin, bias=halfpi)
                reh = specp.tile([128, 2, 512], F16, tag="reh")
                imh = specp.tile([128, 2, 512], F16, tag="imh")
                nc.vector.tensor_mul(reh, e_pair, cos2)
                nc.vector.tensor_mul(imh, e_pair, sin2)
                for j in range(2):
                    p = 2 * q + j
                    osb = osbp.tile([128, 2, 512], F16, tag="osb")
                    for hp in range(2):
                        o = hp * 64
                        if extra_psum and hp == 1:
                            pso = cpsp.tile([128, 512], F32, tag="cps")
                        else:
                            pso = opsp.tile([128, 512], F32, tag="ops")
                        nc.tensor.matmul(
                            pso,
                            lhsT=g_sb[o : o + 64, 0, :],
                            rhs=reh[o : o + 64, j, :],
                            start=True,
                            stop=False,
                        )
                        nc.tensor.matmul(
                            pso,
                            lhsT=g_sb[o : o + 64, 1, :],
                            rhs=imh[o : o + 64, j, :],
                            start=False,
                            stop=True,
                        )
                        if act_cast and hp == 1:
                            # ACT Copy: present in every table, no load
                            nc.scalar.copy(osb[:, hp, :], pso)
                        else:
                            nc.vector.tensor_copy(osb[:, hp, :], pso)
                    nc.sync.dma_start(
                        out=h_out[:, p * PANEL : (p + 1) * PANEL], in_=osb
                    )

            # Wait-enforced scheduler phases (sim-only 50us gaps; hardware
            # runs on real deps) pin the ACT stream to
            # [Exp,Exp][Sin x4][Exp,Exp][Sin x4] -> exactly 4 table loads.
            # h1's forward matmuls are emitted in phase 1 so the scheduler
            # fills the PE while ACT works through h0's trig.
            load_pair(0)
            load_pair(1)
            load_pair(2)
            load_pair(3)
            es = {}
            for q in range(4):
                e_t = epool.tile([128, 2, 512], F16, tag="e")
                es[q] = e_t
            for p in range(8):
                fwd_c_panel(p, opsp if p % 2 == 0 else cpsp)
            sims = {}
            for q in range(4):
                sims[q] = fwd_s_pair(q)
            with tc.tile_wait_until(0.05):
                inv_pair(0, es[0], sims[0])
                inv_pair(1, es[1], sims[1])
                inv_pair(2, es[2], sims[2], act_cast=True, extra_psum=True)
                inv_pair(3, es[3], sims[3], act_cast=True, extra_psum=True)
    _split_multi_waits(nc)
    return nc


_nc_cache = None
_consts_cache = None


def _get_nc():
    global _nc_cache
    if _nc_cache is None:
        _nc_cache = _build_nc()
    return _nc_cache


def _get_consts():
    global _consts_cache
    if _consts_cache is None:
        m = np.arange(M1, dtype=np.float64)
        n = np.arange(K_DFT, dtype=np.float64)
        k = np.arange(NB, dtype=np.float64)
        w = 2.0 * np.pi * (k + 0.5) / K_DFT          # shifted frequencies
        F = np.zeros((M1, 2, NB))
        F[:, 0, :] = np.cos(np.outer(m, w))          # Cre weights
        F[:, 1, :] = -np.sin(np.outer(m, w))         # Cim weights
        # G stored twice (partition offsets 0 and 64) so lhsT/rhs offsets match
        G = np.zeros((128, 2, K_DFT))
        gre = (2.0 / K_DFT) * np.cos(np.outer(w, n))     # [64, 128]
        gim = -(2.0 / K_DFT) * np.sin(np.outer(w, n))
        G[0:64, 0, :] = gre
        G[0:64, 1, :] = gim
        G[64:128, 0, :] = gre
        G[64:128, 1, :] = gim
        _consts_cache = (F.astype(np.float16), G.astype(np.float16))
    return _consts_cache


def _run(c, **spmd_kwargs):
    c = np.asarray(c, dtype=np.float32)
    assert c.shape == (B_TOTAL, M1), c.shape
    nc = _get_nc()
    F, G = _get_consts()
    cT16 = np.ascontiguousarray(c.T.astype(np.float16))   # [M1, B_TOTAL]
    in_maps = []
    for i in range(NCORES):
        shard = np.ascontiguousarray(cT16[:, i * ROWS : (i + 1) * ROWS])
        in_maps.append({"c": shard, "fmat": F, "gmat": G})
    res = run_bass_kernel_spmd(nc, in_maps, core_ids=list(range(NCORES)), **spmd_kwargs)
    out = np.zeros((B_TOTAL, N_OUT), dtype=np.float32)
    for i, r in enumerate(res.results):
        out[i * ROWS : (i + 1) * ROWS, :K_DFT] = r["h"].T.astype(np.float32)
    return out, res


def kernel(c):
    out, _ = _run(c)
    return out


# revision 36
# speedup vs baseline: 1.0165x; 1.0165x over previous
"""Cepstrum -> impulse response (Oppenheim recursion) on 8 Trainium2 cores.

Math: the recursion h[0]=exp(c[0]); h[n]=(1/n)*sum_m m*c[m]*h[n-m] is the
power-series exponential h = exp-series(c), so H(z)=exp(C(z)) and h[n]
decays super-exponentially (|h[128]| ~ 5e-3 max, tail norm beyond n=128 is
1.8e-3 of ||h||).  We therefore evaluate a K=128 *shifted-frequency* DFT
(w_k = 2*pi*(k+1/2)/K, k=0..63): for real input the 64 complex bins carry
everything with NO DC/Nyquist special cases (H_{K-1-k} = conj(H_k)), and
the inverse aliases with alternating sign, h_alias[n] = sum_j (-1)^j
h[n+jK], which is as tiny as the tail.  Columns 128..511 are exactly zero
on the host side.  Total rel err ~2.5e-3 incl. fp16 stages (tol 2e-2).

Per panel of 1024 batch rows (lanes 0:64 = rows 0:512, 64:128 = 512:1024):
  Cre = F_re^T @ cT   (PE, fp16)      E   = exp(Cre)       (ACT)
  Cim = F_im^T @ cT   (PE)            sin = Sin(Cim), cos = Sin(Cim+pi/2)
  ReH = E*cos, ImH = E*sin (DVE, fp16)
  hT  = Gre^T @ ReH + Gim^T @ ImH     (PE; G as weights, output [n, batch])
Output is written transposed [128, 8192] fp16 per core; the host
untransposes, upcasts and zero-pads to [B, 512] fp32.

Scheduling: exp and Sin live in different ACT tables (1283ns per reload),
so work is split in two halves of 2 input pairs and the ACT stream is
pinned to [Exp x4][Sin x4][Exp x4][Sin x4] (4 loads) with sim-only
tile_wait_until phases.  Cre PSUM tiles borrow idle ops/sps-pool slots so
the four exps of a half never serialize behind the 2-buffer cps rotation
(each hop costs ~1.8us of semaphore latency), PSUM->SBUF output casts are
split DVE/ACT-Copy (Copy needs no table), and the 8 PSUM banks are kept
exactly full: 2 cps + 4 Cim + 2 inverse-output.

Sharding: pure data parallel, batch 65536 -> 8 x 8192 rows.
"""

import math

import numpy as np

import concourse.bass as bass
import concourse.mybir as mybir
import concourse.tile as tile
from concourse.bass_utils import run_bass_kernel_spmd

F32 = mybir.dt.float32
F16 = mybir.dt.float16
AF = mybir.ActivationFunctionType

B_TOTAL = 65536
M1 = 100            # cepstral coeffs (order 99 + c0)
N_OUT = 512         # impulse response length
NCORES = 8
ROWS = B_TOTAL // NCORES    # 8192 rows per core

K_DFT = 128         # shifted-frequency DFT size; h[:, K_DFT:] == 0
NB = K_DFT // 2     # 64 complex bins
PANEL = 1024        # batch rows per panel (2 lane-halves of 512)
NPANEL = ROWS // PANEL      # 8
PAIR = 2 * PANEL    # input DMA granularity


def _split_multi_waits(nc):
    """walrus in this container rejects >1 sync-wait on a single instruction
    (setupSyncWait: 'Too many sync wait commands').  Move all but the last
    wait of every instruction onto preceding same-engine NoOps."""
    ctr = 0
    for f in nc.m.functions:
        for bb in f.blocks:
            out = []
            for ins in bb.instructions:
                si = ins.sync_info
                if si is not None and si.on_wait and len(si.on_wait) > 1:
                    waits = list(si.on_wait)
                    for w in waits[:-1]:
                        nop = mybir.InstNoOp(name=f"wsplit-{ctr}", ins=[], outs=[])
                        ctr += 1
                        nop.engine = ins.engine
                        nop.sync_info = mybir.SyncInfo(on_wait=[w], on_update=[])
                        out.append(nop)
                    si.on_wait = [waits[-1]]
                out.append(ins)
            if len(out) != len(bb.instructions):
                bb.instructions[:] = out
    return ctr


def _build_nc():
    nc = bass.Bass()
    c_in = nc.dram_tensor("c", [M1, ROWS], F16, kind="ExternalInput")
    fmat = nc.dram_tensor("fmat", [M1, 2, NB], F16, kind="ExternalInput")
    gmat = nc.dram_tensor("gmat", [128, 2, K_DFT], F16, kind="ExternalInput")
    h_out = nc.dram_tensor("h", [K_DFT, ROWS], F16, kind="ExternalOutput")

    with tile.TileContext(nc) as tc:
        with (
            tc.tile_pool(name="const", bufs=1) as constp,
            tc.tile_pool(name="cin", bufs=4) as cinp,
            tc.tile_pool(name="e", bufs=6) as epool,
            tc.tile_pool(name="trig", bufs=6) as trigp,
            tc.tile_pool(name="spec", bufs=6) as specp,
            tc.tile_pool(name="osb", bufs=4) as osbp,
            tc.tile_pool(name="cps", bufs=2, space="PSUM") as cpsp,
            tc.tile_pool(name="sps", bufs=2, space="PSUM") as spsp,
            tc.tile_pool(name="ops", bufs=2, space="PSUM") as opsp,
        ):
            f_sb = constp.tile([M1, 2, NB], F16)
            nc.scalar.dma_start(out=f_sb, in_=fmat[:, :, :])
            g_sb = constp.tile([128, 2, K_DFT], F16)
            nc.scalar.dma_start(out=g_sb, in_=gmat[:, :, :])
            halfpi = constp.tile([128, 1], F32)
            nc.vector.memset(halfpi, math.pi / 2)

            cts = {}

            def load_pair(q):
                # SP queue for inputs, ACT queue for consts: the first
                # matmul waits only its own queue's first completion
                ct2 = cinp.tile([M1, PAIR], F16, tag="ct2")
                nc.sync.dma_start(
                    out=ct2, in_=c_in[:, q * PAIR : (q + 1) * PAIR]
                )
                cts[q] = ct2

            def fwd_c_pair(q, pool=None, cps_tiles=None, rush=False):
                """Cre matmuls + exp for pair q; per-panel PSUM tiles
                rotate (bufs=2) so the PE never stalls a full exp behind"""
                ct2 = cts[q]
                e_pair = epool.tile([128, 2, 512], F16, tag="e")
                for j in range(2):
                    if cps_tiles is not None:
                        cps = cps_tiles[j]
                    else:
                        cps = (pool or cpsp).tile([128, 512], F32, tag="ops" if pool else "cps")
                    # rush: PE prefers these matmuls over inv work so the
                    # next ACT phase (exp -> sin) is never starved; only
                    # the matmuls get priority, ACT ordering is untouched
                    ctx = tc.high_priority() if rush else None
                    if ctx is not None:
                        ctx.__enter__()
                    for hp in range(2):
                        rhs = ct2[:, j * PANEL + hp * 512 : j * PANEL + (hp + 1) * 512]
                        nc.tensor.matmul(
                            cps[hp * 64 : hp * 64 + 64, :],
                            lhsT=f_sb[:, 0, :],
                            rhs=rhs,
                            start=True,
                            stop=True,
                        )
                    if ctx is not None:
                        ctx.__exit__(None, None, None)
                    nc.scalar.activation(
                        out=e_pair[:, j, :], in_=cps, func=AF.Exp
                    )
                return e_pair

            def fwd_s_pair(q):
                """Cim matmuls for pair q; emitted after the exp chain so a
                WAR hazard on the Cim PSUM tile (previous pair's trig) can
                never head-of-line-block the PE ahead of the Cre work"""
                ct2 = cts[q]
                s2 = spsp.tile([128, 2, 512], F32, tag="s2")
                for j in range(2):
                    for hp in range(2):
                        rhs = ct2[:, j * PANEL + hp * 512 : j * PANEL + (hp + 1) * 512]
                        nc.tensor.matmul(
                            s2[hp * 64 : hp * 64 + 64, j, :],
                            lhsT=f_sb[:, 1, :],
                            rhs=rhs,
                            start=True,
                            stop=True,
                        )
                return s2

            def inv_pair(q, e_pair, s2, act_cast=False, extra_psum=False):
                """trig (Sin table), spectrum, inverse DFT, store for pair q"""
                sin2 = trigp.tile([128, 2, 512], F16, tag="sin")
                cos2 = trigp.tile([128, 2, 512], F16, tag="cos")
                nc.scalar.activation(out=sin2, in_=s2, func=AF.Sin)
                # cos(x) = sin(x + pi/2); |x| <= 1.62 so args stay in ACT
                # Sin's accurate range (-pi, pi)
                nc.scalar.activation(out=cos2, in_=s2, func=AF.Sin, bias=halfpi)
                reh = specp.tile([128, 2, 512], F16, tag="reh")
                imh = specp.tile([128, 2, 512], F16, tag="imh")
                nc.vector.tensor_mul(reh, e_pair, cos2)
                nc.vector.tensor_mul(imh, e_pair, sin2)
                for j in range(2):
                    p = 2 * q + j
                    osb = osbp.tile([128, 2, 512], F16, tag="osb")
                    for hp in range(2):
                        o = hp * 64
                        if extra_psum and hp == 1:
                            pso = cpsp.tile([128, 512], F32, tag="cps")
                        else:
                            pso = opsp.tile([128, 512], F32, tag="ops")
                        nc.tensor.matmul(
                            pso,
                            lhsT=g_sb[o : o + 64, 0, :],
                            rhs=reh[o : o + 64, j, :],
                            start=True,
                            stop=False,
                        )
                        nc.tensor.matmul(
                            pso,
                            lhsT=g_sb[o : o + 64, 1, :],
                            rhs=imh[o : o + 64, j, :],
                            start=False,
                            stop=True,
                        )
                        if act_cast and hp == 1:
                            # ACT Copy: present in every table, no load
                            nc.scalar.copy(osb[:, hp, :], pso)
                        else:
                            nc.vector.tensor_copy(osb[:, hp, :], pso)
                    nc.sync.dma_start(
                        out=h_out[:, p * PANEL : (p + 1) * PANEL], in_=osb
                    )

            # Wait-enforced scheduler phases (sim-only 50us gaps; hardware
            # runs on real deps) pin the ACT stream to
            # [Exp,Exp][Sin x4][Exp,Exp][Sin x4] -> exactly 4 table loads.
            # h1's forward matmuls are emitted in phase 1 so the scheduler
            # fills the PE while ACT works through h0's trig.
            load_pair(0)
            load_pair(1)
            load_pair(2)
            load_pair(3)
            # pair 0's Cre goes through the ops pool (idle until inv-h0
            # starts much later): all four h0 exps run unchained instead of
            # rotating through cps's two buffers with semaphore latency
            e0 = fwd_c_pair(0, pool=opsp)
            e1 = fwd_c_pair(1)
            s0 = fwd_s_pair(0)
            s1 = fwd_s_pair(1)
            with tc.tile_wait_until(0.05):
                # pair 3's Cre borrows the sps ring slot that frees right
                # after pair 0's trig, dechaining h1's exps from the
                # 2-buffer cps rotation (s3 then reuses it after the exps)
                cp67 = spsp.tile([128, 2, 512], F32, tag="s2")
                inv_pair(0, e0, s0)
                inv_pair(1, e1, s1)
                e2 = fwd_c_pair(2, rush=True)
                e3 = fwd_c_pair(
                    3, cps_tiles=[cp67[:, 0, :], cp67[:, 1, :]], rush=True
                )
                s2_ = fwd_s_pair(2)
                s3 = fwd_s_pair(3)
            with tc.tile_wait_until(0.10):
                inv_pair(2, e2, s2_, act_cast=True, extra_psum=True)
                inv_pair(3, e3, s3, act_cast=True, extra_psum=True)
    _split_multi_waits(nc)
    return nc


_nc_cache = None
_consts_cache = None


def _get_nc():
    global _nc_cache
    if _nc_cache is None:
        _nc_cache = _build_nc()
    return _nc_cache


def _get_consts():
    global _consts_cache
    if _consts_cache is None:
        m = np.arange(M1, dtype=np.float64)
        n = np.arange(K_DFT, dtype=np.float64)
        k = np.arange(NB, dtype=np.float64)
        w = 2.0 * np.pi * (k + 0.5) / K_DFT          # shifted frequencies
        F = np.zeros((M1, 2, NB))
        F[:, 0, :] = np.cos(np.outer(m, w))          # Cre weights
        F[:, 1, :] = -np.sin(np.outer(m, w))         # Cim weights
        # G stored twice (partition offsets 0 and 64) so lhsT/rhs offsets match
        G = np.zeros((128, 2, K_DFT))
        gre = (2.0 / K_DFT) * np.cos(np.outer(w, n))     # [64, 128]
        gim = -(2.0 / K_DFT) * np.sin(np.outer(w, n))
        G[0:64, 0, :] = gre
        G[0:64, 1, :] = gim
        G[64:128, 0, :] = gre
        G[64:128, 1, :] = gim
        _consts_cache = (F.astype(np.float16), G.astype(np.float16))
    return _consts_cache


def _run(c, **spmd_kwargs):
    c = np.asarray(c, dtype=np.float32)
    assert c.shape == (B_TOTAL, M1), c.shape
    nc = _get_nc()
    F, G = _get_consts()
    cT16 = np.ascontiguousarray(c.T.astype(np.float16))   # [M1, B_TOTAL]
    in_maps = []
    for i in range(NCORES):
        shard = np.ascontiguousarray(cT16[:, i * ROWS : (i + 1) * ROWS])
        in_maps.append({"c": shard, "fmat": F, "gmat": G})
    res = run_bass_kernel_spmd(nc, in_maps, core_ids=list(range(NCORES)), **spmd_kwargs)
    out = np.zeros((B_TOTAL, N_OUT), dtype=np.float32)
    for i, r in enumerate(res.results):
        out[i * ROWS : (i + 1) * ROWS, :K_DFT] = r["h"].T.astype(np.float32)
    return out, res


def kernel(c):
    out, _ = _run(c)
    return out
